# revision 1
# baseline (speedup 1.0000x reference)
"""StyleGAN2 fused upsample2x + 3x3 conv + FIR(1,3,3,1) + bias — TRN2 Bass kernel v2.

Unlike v1 (which folded the FIR into the conv weights, 4x the matmul work),
this version computes the four parity planes of the stride-2 transposed conv
directly (9 taps total across planes -> 4x fewer MACs on TensorE), then applies
the separable FIR (1,3,3,1)/4 per dimension as fused scalar_tensor_tensor ops:

  y parity planes (PSUM, fp32) --ScalarE copy--> bf16 SBUF (plus shifted-by-1
  copies via GPSIMD so every DVE operand stays 4B-aligned => 2x bf16 mode)
  --DVE col FIR--> h planes --DVE row FIR--> quadrants
  --ScalarE scale(1/16)+bias--> interleaved fp32 out --DMA--> HBM.

Data-parallel over batch: 2 images per core, 8 cores.  Matmuls in float32r.
"""

import sys

sys.path.insert(0, "/opt/trn_rl_repo")

import numpy as np

import concourse.bacc as bacc
import concourse.mybir as mybir
import concourse.tile as tile
from concourse.bass_utils import run_bass_kernel_spmd

N_CORES = 8
IMGS = 16
IMG_PER_CORE = IMGS // N_CORES  # 2
C = 256
O = 256
H = W = 64
NK = C // 128  # contraction splits
NM = O // 128  # output-channel splits
XR, XC = H + 3, W + 4  # padded input rows/cols (67, 68)
PW = W + 2  # stored plane width (66)

# (plane, rows, [(du, dv, wi, wj), ...]) in kernel iteration order.
# E-class planes have H+1 rows, O-class (row-shifted storage) H+2.
PLANES = [
    ("Ee", H + 1, [(0, 0, 0, 0), (0, 1, 0, 2), (1, 0, 2, 0), (1, 1, 2, 2)]),
    ("Eo", H + 1, [(0, 0, 0, 1), (1, 0, 2, 1)]),
    ("Oe", H + 2, [(0, 0, 1, 0), (0, 1, 1, 2)]),
    ("Oo", H + 2, [(0, 0, 1, 1)]),
]


def _chunks(rows):
    """7-row PSUM chunks paired into bands of (up to) 14 rows."""
    starts = list(range(0, rows, 7))
    ch = [(s, min(7, rows - s)) for s in starts]
    bands = [ch[i : i + 2] for i in range(0, len(ch), 2)]
    return bands


def _stat_order():
    """Stationary weight order: (m, plane_idx, tap_idx, k) -> flat index."""
    order = []
    for m in range(NM):
        for pi, (_, _, taps) in enumerate(PLANES):
            for ti in range(len(taps)):
                for k in range(NK):
                    order.append((m, pi, ti, k))
    return {key: i for i, key in enumerate(order)}


STAT_IDX = _stat_order()
NSTAT = len(STAT_IDX)  # 36
NSTAT_TOT = NSTAT + 2  # + identity, 3*identity for FIR combine matmuls

_compiled = None
LAST_RESULTS = None


def _build():
    nc = bacc.Bacc(None, target_bir_lowering=False, debug=False)
    dt = mybir.dt
    f32r, f32, bf16 = dt.float32r, dt.float32, dt.bfloat16
    MULT, ADD = mybir.AluOpType.mult, mybir.AluOpType.add

    xp_d = nc.dram_tensor(
        "xp", (IMG_PER_CORE, NK, 128, XR * XC), bf16, kind="ExternalInput"
    )
    wt_d = nc.dram_tensor("wt", (128, NSTAT_TOT * 128), bf16, kind="ExternalInput")
    b_d = nc.dram_tensor("bias", (128, NM), f32, kind="ExternalInput")
    out_d = nc.dram_tensor(
        "out", (IMG_PER_CORE, O, 2 * H, 2 * W), f32, kind="ExternalOutput"
    )

    with tile.TileContext(nc) as tc:
        with (
            tc.tile_pool(name="xpool", bufs=1) as xpool,
            tc.tile_pool(name="wpool", bufs=1) as wpool,
            tc.tile_pool(name="ybpool", bufs=4) as ybpool,
            tc.tile_pool(name="pqpool", bufs=2) as pqpool,
            tc.tile_pool(name="hpool", bufs=1) as hpool,
            tc.tile_pool(name="cpool", bufs=2) as cpool,
            tc.tile_pool(name="qpool", bufs=6) as qpool,
            tc.tile_pool(name="opool", bufs=2) as opool,
            tc.tile_pool(name="psum", bufs=6, space="PSUM") as psum_pool,
            tc.tile_pool(name="hpsum", bufs=2, space="PSUM") as hpsum,
        ):
            wt_t = wpool.tile([128, NSTAT_TOT * 128], bf16, tag="wt")
            b_t = wpool.tile([128, NM], f32, tag="bias")
            xp_t = {}

            def load_xp(img, k, split=False):
                t = xpool.tile([128, XR, XC], bf16, tag=f"xp{img}{k}")
                src = xp_d.ap()[img, k].rearrange("p (h w) -> p h w", h=XR)
                if split:
                    nc.sync.dma_start(t[:, :20, :], src[:, :20, :])
                    nc.sync.dma_start(t[:, 20:, :], src[:, 20:, :])
                else:
                    nc.sync.dma_start(t[:], src)
                xp_t[img, k] = t

            # Minimal working set first: weights for (m0, plane Ee), first xp
            # rows, then the rest.
            nc.sync.dma_start(wt_t[:, : 8 * 128], wt_d.ap()[:, : 8 * 128])
            load_xp(0, 0, split=True)
            nc.sync.dma_start(b_t[:], b_d.ap()[:])
            load_xp(0, 1, split=True)
            nc.sync.dma_start(wt_t[:, 8 * 128 :], wt_d.ap()[:, 8 * 128 :])
            load_xp(1, 0)
            load_xp(1, 1)

            for img in range(IMG_PER_CORE):
                for m in range(NM):
                    # ---- stage A+B: matmul parity planes, evac, col FIR ----
                    h_t = {}
                    for name, rows, _ in PLANES:
                        h_t[name] = hpool.tile(
                            [128, rows, W], bf16, tag=f"h{name}", name=f"h{name}"
                        )

                    for cls, rows in (("E", H + 1), ("O", H + 2)):
                        pe_i, po_i = (0, 1) if cls == "E" else (2, 3)
                        _, _, pe_taps = PLANES[pe_i]
                        _, _, po_taps = PLANES[po_i]
                        for band in _chunks(rows):
                            r0 = band[0][0]
                            nr = sum(n for _, n in band)
                            psums = {}
                            for pi, taps in ((pe_i, pe_taps), (po_i, po_taps)):
                                pts = [
                                    psum_pool.tile(
                                        [128, n, PW], f32, tag="ps", name="ps"
                                    )
                                    for _, n in band
                                ]
                                n_ops = len(taps) * NK
                                acc = 0
                                for ti, (du, dv, _, _) in enumerate(taps):
                                    for k in range(NK):
                                        si = STAT_IDX[(m, pi, ti, k)]
                                        lhsT = wt_t[:, si * 128 : (si + 1) * 128]
                                        for ci, (cs, cn) in enumerate(band):
                                            rhs = xp_t[img, k][
                                                :,
                                                cs + du : cs + du + cn,
                                                dv : dv + PW,
                                            ]
                                            nc.tensor.matmul(
                                                pts[ci][:],
                                                lhsT,
                                                rhs,
                                                start=(acc == 0),
                                                stop=(acc == n_ops - 1),
                                            )
                                        acc += 1
                                psums[pi] = pts

                            # evac: n copies only (ScalarE, PSUM->SBUF bf16)
                            yb = {}
                            for pi in (pe_i, po_i):
                                n_t = ybpool.tile([128, nr, PW], bf16, tag="ybn")
                                ro = 0
                                for ci, (cs, cn) in enumerate(band):
                                    nc.scalar.copy(
                                        n_t[:, ro : ro + cn, :], psums[pi][ci][:]
                                    )
                                    ro += cn
                                yb[pi] = n_t

                            ne = yb[pe_i]
                            no = yb[po_i]
                            # col FIR (unnormalized x4), S/T form:
                            #   hRe = 3*(ne[0]+no[1]) + (ne[1]+no[0])
                            #   hRo = 3*(ne[1]+no[1]) + (ne[0]+no[2])
                            S1 = pqpool.tile([128, nr, W], bf16, tag="S1")
                            T1 = pqpool.tile([128, nr, W], bf16, tag="T1")
                            S2 = pqpool.tile([128, nr, PW], bf16, tag="S2")
                            T2 = pqpool.tile([128, nr, W], bf16, tag="T2")
                            nc.gpsimd.tensor_tensor(
                                S1[:], ne[:, :, 0:W], no[:, :, 1 : W + 1], ADD
                            )
                            nc.gpsimd.tensor_tensor(
                                T1[:], ne[:, :, 1 : W + 1], no[:, :, 0:W], ADD
                            )
                            nc.vector.tensor_tensor(S2[:], ne[:], no[:], ADD)
                            nc.vector.tensor_tensor(
                                T2[:], ne[:, :, 0:W], no[:, :, 2 : W + 2], ADD
                            )
                            he = h_t["Ee" if cls == "E" else "Oe"]
                            ho = h_t["Eo" if cls == "E" else "Oo"]
                            hps = [
                                hpsum.tile([128, cn, W], f32, tag="hps", name="hps")
                                for _, cn in band
                            ]
                            I3 = wt_t[:, (NSTAT + 1) * 128 : (NSTAT + 2) * 128]
                            I1 = wt_t[:, NSTAT * 128 : (NSTAT + 1) * 128]
                            ro = 0
                            for ci, (cs, cn) in enumerate(band):
                                nc.tensor.matmul(
                                    hps[ci][:], I3, S1[:, ro : ro + cn, :],
                                    start=True, stop=False,
                                )
                                ro += cn
                            ro = 0
                            for ci, (cs, cn) in enumerate(band):
                                nc.tensor.matmul(
                                    hps[ci][:], I1, T1[:, ro : ro + cn, :],
                                    start=False, stop=True,
                                )
                                ro += cn
                            ro = 0
                            for ci, (cs, cn) in enumerate(band):
                                nc.scalar.copy(
                                    he[:, r0 + ro : r0 + ro + cn, :], hps[ci][:]
                                )
                                ro += cn
                            nc.vector.scalar_tensor_tensor(
                                ho[:, r0 : r0 + nr, :],
                                S2[:, :, 1 : W + 1], 3.0, T2[:], MULT, ADD
                            )

                    # ---- stage C: row FIR (banded) + final ----
                    for qb in range(4):
                        u0 = qb * 16
                        quads = {}
                        for cp in ("e", "o"):
                            A = h_t["E" + cp]
                            B = h_t["O" + cp]
                            A3 = cpool.tile([128, 17, W], bf16, tag="A3")
                            B3 = cpool.tile([128, 18, W], bf16, tag="B3")
                            nc.vector.tensor_scalar_mul(
                                A3[:], A[:, u0 : u0 + 17, :], 3.0
                            )
                            nc.vector.tensor_scalar_mul(
                                B3[:], B[:, u0 : u0 + 18, :], 3.0
                            )
                            # oE = (3A[u] + A[u+1]) + (B[u] + 3B[u+1])
                            X1 = cpool.tile([128, 16, W], bf16, tag="X1")
                            Y1 = cpool.tile([128, 16, W], bf16, tag="Y1")
                            nc.vector.tensor_tensor(
                                X1[:], A3[:, 0:16, :], A[:, u0 + 1 : u0 + 17, :], ADD
                            )
                            nc.vector.tensor_tensor(
                                Y1[:], B[:, u0 : u0 + 16, :], B3[:, 1:17, :], ADD
                            )
                            oE = qpool.tile([128, 16, W], bf16, tag="oE")
                            nc.vector.tensor_tensor(oE[:], X1[:], Y1[:], ADD)
                            # oO = (A[u] + 3A[u+1]) + (3B[u+1] + B[u+2])
                            X2 = cpool.tile([128, 16, W], bf16, tag="X2")
                            Y2 = cpool.tile([128, 16, W], bf16, tag="Y2")
                            nc.vector.tensor_tensor(
                                X2[:], A[:, u0 : u0 + 16, :], A3[:, 1:17, :], ADD
                            )
                            nc.vector.tensor_tensor(
                                Y2[:], B3[:, 1:17, :], B[:, u0 + 2 : u0 + 18, :], ADD
                            )
                            oO = qpool.tile([128, 16, W], bf16, tag="oO")
                            nc.vector.tensor_tensor(oO[:], X2[:], Y2[:], ADD)
                            quads[(0, cp)] = oE
                            quads[(1, cp)] = oO

                        out_sb = opool.tile([128, 32, 2 * W], f32)
                        for alpha in range(2):
                            for cj, cp in enumerate(("e", "o")):
                                nc.scalar.activation(
                                    out_sb[:, alpha::2, cj::2],
                                    quads[(alpha, cp)][:],
                                    mybir.ActivationFunctionType.Identity,
                                    bias=b_t[:, m : m + 1],
                                    scale=1.0 / 16.0,
                                )
                        nc.sync.dma_start(
                            out_d.ap()[
                                img,
                                m * 128 : (m + 1) * 128,
                                qb * 32 : (qb + 1) * 32,
                                :,
                            ],
                            out_sb[:],
                        )

    nc.compile()
    return nc


def _prep_weights(w):
    """w (256,256,3,3) -> [c_local, (stat idx, o_local)] fp32."""
    wt = np.empty((128, NSTAT_TOT, 128), dtype=np.float32)
    wt[:, NSTAT, :] = np.eye(128, dtype=np.float32)
    wt[:, NSTAT + 1, :] = 3.0 * np.eye(128, dtype=np.float32)
    for m in range(NM):
        for pi, (_, _, taps) in enumerate(PLANES):
            for ti, (_, _, wi, wj) in enumerate(taps):
                for k in range(NK):
                    si = STAT_IDX[(m, pi, ti, k)]
                    sub = w[m * 128 : (m + 1) * 128, k * 128 : (k + 1) * 128, wi, wj]
                    wt[:, si, :] = sub.T
    return np.ascontiguousarray(wt.reshape(128, NSTAT_TOT * 128))


def kernel(x, w, b):
    global _compiled, LAST_RESULTS
    if _compiled is None:
        _compiled = _build()
    nc = _compiled

    x = np.asarray(x, dtype=np.float32)
    w = np.asarray(w, dtype=np.float32)
    b = np.asarray(b, dtype=np.float32)

    import ml_dtypes

    wt = _prep_weights(w).astype(ml_dtypes.bfloat16)
    b2 = np.ascontiguousarray(b.reshape(NM, 128).T)
    xp = np.zeros((IMGS, C, XR, XC), dtype=np.float32)
    xp[:, :, 1 : H + 1, 1 : W + 1] = x
    xp = np.ascontiguousarray(
        xp.reshape(N_CORES, IMG_PER_CORE, NK, 128, XR * XC)
    ).astype(ml_dtypes.bfloat16)

    in_maps = [
        {"xp": xp[core], "wt": wt, "bias": b2} for core in range(N_CORES)
    ]
    try:
        res = run_bass_kernel_spmd(nc, in_maps, list(range(N_CORES)))
    except ModuleNotFoundError:
        import os

        os.environ["BASS_NEVER_TRACE"] = "1"
        res = run_bass_kernel_spmd(nc, in_maps, list(range(N_CORES)))
    LAST_RESULTS = res
    out = np.concatenate([res.results[i]["out"] for i in range(N_CORES)], axis=0)
    return out



# revision 3
# speedup vs baseline: 1.3378x; 1.3378x over previous
"""StyleGAN2 fused upsample2x + 3x3 conv + FIR(1,3,3,1) + bias — TRN2 Bass kernel v2.

Unlike v1 (which folded the FIR into the conv weights, 4x the matmul work),
this version computes the four parity planes of the stride-2 transposed conv
directly (9 taps total across planes -> 4x fewer MACs on TensorE), then applies
the separable FIR (1,3,3,1)/4 per dimension as fused scalar_tensor_tensor ops:

  y parity planes (PSUM, fp32) --ScalarE copy--> bf16 SBUF (plus shifted-by-1
  copies via GPSIMD so every DVE operand stays 4B-aligned => 2x bf16 mode)
  --DVE col FIR--> h planes --DVE row FIR--> quadrants
  --ScalarE scale(1/16)+bias--> interleaved fp32 out --DMA--> HBM.

Data-parallel over batch: 2 images per core, 8 cores.  Matmuls in float32r.
"""

import sys

sys.path.insert(0, "/opt/trn_rl_repo")

import numpy as np

import concourse.bacc as bacc
import concourse.mybir as mybir
import concourse.tile as tile
from concourse.bass_utils import run_bass_kernel_spmd

N_CORES = 8
IMGS = 16
IMG_PER_CORE = IMGS // N_CORES  # 2
C = 256
O = 256
H = W = 64
NK = C // 128  # contraction splits
NM = O // 128  # output-channel splits
XR, XC = H + 3, W + 4  # padded input rows/cols (67, 68)
PW = W + 2  # stored plane width (66)

# (plane, rows, [(du, dv, wi, wj), ...]) in kernel iteration order.
# E-class planes have H+1 rows, O-class (row-shifted storage) H+2.
PLANES = [
    ("Ee", H + 1, [(0, 0, 0, 0), (0, 1, 0, 2), (1, 0, 2, 0), (1, 1, 2, 2)]),
    ("Eo", H + 1, [(0, 0, 0, 1), (1, 0, 2, 1)]),
    ("Oe", H + 2, [(0, 0, 1, 0), (0, 1, 1, 2)]),
    ("Oo", H + 2, [(0, 0, 1, 1)]),
]


def _chunks(rows):
    """7-row PSUM chunks paired into bands of (up to) 14 rows."""
    starts = list(range(0, rows, 7))
    ch = [(s, min(7, rows - s)) for s in starts]
    bands = [ch[i : i + 2] for i in range(0, len(ch), 2)]
    return bands


def _stat_order():
    """Stationary weight order: (m, plane_idx, tap_idx, k) -> flat index."""
    order = []
    for m in range(NM):
        for pi, (_, _, taps) in enumerate(PLANES):
            for ti in range(len(taps)):
                for k in range(NK):
                    order.append((m, pi, ti, k))
    return {key: i for i, key in enumerate(order)}


STAT_IDX = _stat_order()
NSTAT = len(STAT_IDX)  # 36
NSTAT_TOT = NSTAT + 2  # + identity, 3*identity for FIR combine matmuls

_compiled = None
LAST_RESULTS = None


def _build():
    nc = bacc.Bacc(None, target_bir_lowering=False, debug=False)
    dt = mybir.dt
    f32r, f32, bf16 = dt.float32r, dt.float32, dt.bfloat16
    MULT, ADD = mybir.AluOpType.mult, mybir.AluOpType.add

    xp_d = nc.dram_tensor(
        "xp", (IMG_PER_CORE, NK, 128, XR * XC), bf16, kind="ExternalInput"
    )
    wt_d = nc.dram_tensor("wt", (128, NSTAT_TOT * 128), bf16, kind="ExternalInput")
    b_d = nc.dram_tensor("bias", (128, NM), f32, kind="ExternalInput")
    out_d = nc.dram_tensor(
        "out", (IMG_PER_CORE, O, 2 * H, 2 * W), f32, kind="ExternalOutput"
    )

    with tile.TileContext(nc) as tc:
        with (
            tc.tile_pool(name="xpool", bufs=1) as xpool,
            tc.tile_pool(name="wpool", bufs=1) as wpool,
            tc.tile_pool(name="ybpool", bufs=4) as ybpool,
            tc.tile_pool(name="pqpool", bufs=2) as pqpool,
            tc.tile_pool(name="hpool", bufs=2) as hpool,
            tc.tile_pool(name="cpool", bufs=2) as cpool,
            tc.tile_pool(name="qpool", bufs=4) as qpool,
            tc.tile_pool(name="opool", bufs=2) as opool,
            tc.tile_pool(name="psum", bufs=6, space="PSUM") as psum_pool,
            tc.tile_pool(name="qpsum", bufs=2, space="PSUM") as qpsum,
        ):
            wt_t = wpool.tile([128, NSTAT_TOT * 128], bf16, tag="wt")
            b_t = wpool.tile([128, NM], f32, tag="bias")
            xp_t = {}

            def load_xp(img, k, split=False):
                t = xpool.tile([128, XR, XC], bf16, tag=f"xp{img}{k}")
                src = xp_d.ap()[img, k].rearrange("p (h w) -> p h w", h=XR)
                if split:
                    nc.sync.dma_start(t[:, :20, :], src[:, :20, :])
                    nc.sync.dma_start(t[:, 20:, :], src[:, 20:, :])
                else:
                    nc.sync.dma_start(t[:], src)
                xp_t[img, k] = t

            # Minimal working set first: weights for (m0, plane Ee), first xp
            # rows, then the rest.
            nc.sync.dma_start(wt_t[:, : 8 * 128], wt_d.ap()[:, : 8 * 128])
            load_xp(0, 0, split=True)
            nc.sync.dma_start(b_t[:], b_d.ap()[:])
            load_xp(0, 1, split=True)
            nc.sync.dma_start(wt_t[:, 8 * 128 :], wt_d.ap()[:, 8 * 128 :])
            load_xp(1, 0)
            load_xp(1, 1)

            I1 = wt_t[:, NSTAT * 128 : (NSTAT + 1) * 128]
            I3 = wt_t[:, (NSTAT + 1) * 128 : (NSTAT + 2) * 128]

            for img in range(IMG_PER_CORE):
                for m in range(NM):
                    # ---- stage A+B: matmul parity planes, evac, col FIR ----
                    h_t = {}
                    for name, rows, _ in PLANES:
                        h_t[name] = hpool.tile(
                            [128, rows, W], bf16, tag=f"h{name}", name=f"h{name}"
                        )

                    for cls, rows in (("E", H + 1), ("O", H + 2)):
                        pe_i, po_i = (0, 1) if cls == "E" else (2, 3)
                        _, _, pe_taps = PLANES[pe_i]
                        _, _, po_taps = PLANES[po_i]
                        for band in _chunks(rows):
                            r0 = band[0][0]
                            nr = sum(n for _, n in band)
                            psums = {}
                            for pi, taps in ((pe_i, pe_taps), (po_i, po_taps)):
                                pts = [
                                    psum_pool.tile(
                                        [128, n, PW], f32, tag="ps", name="ps"
                                    )
                                    for _, n in band
                                ]
                                n_ops = len(taps) * NK
                                acc = 0
                                for ti, (du, dv, _, _) in enumerate(taps):
                                    for k in range(NK):
                                        si = STAT_IDX[(m, pi, ti, k)]
                                        lhsT = wt_t[:, si * 128 : (si + 1) * 128]
                                        for ci, (cs, cn) in enumerate(band):
                                            rhs = xp_t[img, k][
                                                :,
                                                cs + du : cs + du + cn,
                                                dv : dv + PW,
                                            ]
                                            nc.tensor.matmul(
                                                pts[ci][:],
                                                lhsT,
                                                rhs,
                                                start=(acc == 0),
                                                stop=(acc == n_ops - 1),
                                            )
                                        acc += 1
                                psums[pi] = pts

                            # evac: n copies only (ScalarE, PSUM->SBUF bf16)
                            yb = {}
                            for pi in (pe_i, po_i):
                                n_t = ybpool.tile([128, nr, PW], bf16, tag="ybn")
                                ro = 0
                                for ci, (cs, cn) in enumerate(band):
                                    nc.scalar.copy(
                                        n_t[:, ro : ro + cn, :], psums[pi][ci][:]
                                    )
                                    ro += cn
                                yb[pi] = n_t

                            ne = yb[pe_i]
                            no = yb[po_i]
                            # col FIR (unnormalized x4), all on DVE:
                            #   he = 3*ne[0] + ne[1] + 3*no[1] + no[0]
                            #      = (3*ne[0]+no[0]) + (3*no[1]+ne[1])
                            #   ho = 3*ne[1] + ne[0] + 3*no[1] + no[2]
                            #      = 3*(ne[1]+no[1]) + (ne[0]+no[2])
                            he = h_t["Ee" if cls == "E" else "Oe"]
                            ho = h_t["Eo" if cls == "E" else "Oo"]
                            A_ = pqpool.tile([128, nr, W], bf16, tag="Ac")
                            B_ = pqpool.tile([128, nr, W], bf16, tag="Bc")
                            U_ = pqpool.tile([128, nr, W], bf16, tag="Uc")
                            T_ = pqpool.tile([128, nr, W], bf16, tag="Tc")
                            nc.vector.scalar_tensor_tensor(
                                A_[:], ne[:, :, 0:W], 3.0, no[:, :, 0:W], MULT, ADD
                            )
                            nc.vector.scalar_tensor_tensor(
                                B_[:], no[:, :, 1 : W + 1], 3.0,
                                ne[:, :, 1 : W + 1], MULT, ADD
                            )
                            nc.vector.tensor_tensor(
                                he[:, r0 : r0 + nr, :], A_[:], B_[:], ADD
                            )
                            nc.vector.tensor_tensor(
                                U_[:], ne[:, :, 0:W], no[:, :, 2 : W + 2], ADD
                            )
                            nc.vector.tensor_tensor(
                                T_[:], ne[:, :, 1 : W + 1], no[:, :, 1 : W + 1], ADD
                            )
                            nc.vector.scalar_tensor_tensor(
                                ho[:, r0 : r0 + nr, :], T_[:], 3.0, U_[:], MULT, ADD
                            )

                    # ---- stage C: row FIR + final ----
                    # even out-rows (alpha=0) on DVE, odd (alpha=1) on TensorE
                    for qb in range(4):
                        u0 = qb * 16
                        out_sb = opool.tile([128, 32, 2 * W], f32)
                        for cj, cp in enumerate(("e", "o")):
                            A = h_t["E" + cp]
                            B = h_t["O" + cp]
                            # oE = (3A[u] + A[u+1]) + (3B[u+1] + B[u])
                            P_ = cpool.tile([128, 16, W], bf16, tag="Pr")
                            Q_ = cpool.tile([128, 16, W], bf16, tag="Qr")
                            nc.vector.scalar_tensor_tensor(
                                P_[:], A[:, u0 : u0 + 16, :], 3.0,
                                A[:, u0 + 1 : u0 + 17, :], MULT, ADD
                            )
                            nc.vector.scalar_tensor_tensor(
                                Q_[:], B[:, u0 + 1 : u0 + 17, :], 3.0,
                                B[:, u0 : u0 + 16, :], MULT, ADD
                            )
                            oE = qpool.tile([128, 16, W], bf16, tag="oE")
                            nc.vector.tensor_tensor(oE[:], P_[:], Q_[:], ADD)
                            nc.scalar.activation(
                                out_sb[:, 0::2, cj::2],
                                oE[:],
                                mybir.ActivationFunctionType.Identity,
                                bias=b_t[:, m : m + 1],
                                scale=1.0 / 16.0,
                            )
                            # oO = A[u] + 3A[u+1] + 3B[u+1] + B[u+2]  (PE psum)
                            for sub in range(2):
                                u1 = u0 + sub * 8
                                qp = qpsum.tile([128, 8, W], f32, tag="qp",
                                                name="qp")
                                nc.tensor.matmul(
                                    qp[:], I1, A[:, u1 : u1 + 8, :],
                                    start=True, stop=False,
                                )
                                nc.tensor.matmul(
                                    qp[:], I3, A[:, u1 + 1 : u1 + 9, :],
                                    start=False, stop=False,
                                )
                                nc.tensor.matmul(
                                    qp[:], I3, B[:, u1 + 1 : u1 + 9, :],
                                    start=False, stop=False,
                                )
                                nc.tensor.matmul(
                                    qp[:], I1, B[:, u1 + 2 : u1 + 10, :],
                                    start=False, stop=True,
                                )
                                nc.scalar.activation(
                                    out_sb[:, sub * 16 + 1 : sub * 16 + 16 : 2,
                                           cj::2],
                                    qp[:],
                                    mybir.ActivationFunctionType.Identity,
                                    bias=b_t[:, m : m + 1],
                                    scale=1.0 / 16.0,
                                )
                        nc.sync.dma_start(
                            out_d.ap()[
                                img,
                                m * 128 : (m + 1) * 128,
                                qb * 32 : (qb + 1) * 32,
                                :,
                            ],
                            out_sb[:],
                        )

    nc.compile()
    return nc


def _prep_weights(w):
    """w (256,256,3,3) -> [c_local, (stat idx, o_local)] fp32."""
    wt = np.empty((128, NSTAT_TOT, 128), dtype=np.float32)
    wt[:, NSTAT, :] = np.eye(128, dtype=np.float32)
    wt[:, NSTAT + 1, :] = 3.0 * np.eye(128, dtype=np.float32)
    for m in range(NM):
        for pi, (_, _, taps) in enumerate(PLANES):
            for ti, (_, _, wi, wj) in enumerate(taps):
                for k in range(NK):
                    si = STAT_IDX[(m, pi, ti, k)]
                    sub = w[m * 128 : (m + 1) * 128, k * 128 : (k + 1) * 128, wi, wj]
                    wt[:, si, :] = sub.T
    return np.ascontiguousarray(wt.reshape(128, NSTAT_TOT * 128))


def kernel(x, w, b):
    global _compiled, LAST_RESULTS
    if _compiled is None:
        _compiled = _build()
    nc = _compiled

    x = np.asarray(x, dtype=np.float32)
    w = np.asarray(w, dtype=np.float32)
    b = np.asarray(b, dtype=np.float32)

    import ml_dtypes

    wt = _prep_weights(w).astype(ml_dtypes.bfloat16)
    b2 = np.ascontiguousarray(b.reshape(NM, 128).T)
    xp = np.zeros((IMGS, C, XR, XC), dtype=np.float32)
    xp[:, :, 1 : H + 1, 1 : W + 1] = x
    xp = np.ascontiguousarray(
        xp.reshape(N_CORES, IMG_PER_CORE, NK, 128, XR * XC)
    ).astype(ml_dtypes.bfloat16)

    in_maps = [
        {"xp": xp[core], "wt": wt, "bias": b2} for core in range(N_CORES)
    ]
    try:
        res = run_bass_kernel_spmd(nc, in_maps, list(range(N_CORES)))
    except ModuleNotFoundError:
        import os

        os.environ["BASS_NEVER_TRACE"] = "1"
        res = run_bass_kernel_spmd(nc, in_maps, list(range(N_CORES)))
    LAST_RESULTS = res
    out = np.concatenate([res.results[i]["out"] for i in range(N_CORES)], axis=0)
    return out



# revision 6
# speedup vs baseline: 1.4306x; 1.0694x over previous
"""StyleGAN2 fused upsample2x + 3x3 conv + FIR(1,3,3,1) + bias — TRN2 Bass kernel v2.

Unlike v1 (which folded the FIR into the conv weights, 4x the matmul work),
this version computes the four parity planes of the stride-2 transposed conv
directly (9 taps total across planes -> 4x fewer MACs on TensorE), then applies
the separable FIR (1,3,3,1)/4 per dimension as fused scalar_tensor_tensor ops:

  y parity planes (PSUM, fp32) --ScalarE copy--> bf16 SBUF (plus shifted-by-1
  copies via GPSIMD so every DVE operand stays 4B-aligned => 2x bf16 mode)
  --DVE col FIR--> h planes --DVE row FIR--> quadrants
  --ScalarE scale(1/16)+bias--> interleaved fp32 out --DMA--> HBM.

Data-parallel over batch: 2 images per core, 8 cores.  Matmuls in float32r.
"""

import sys

sys.path.insert(0, "/opt/trn_rl_repo")

import numpy as np

import concourse.bacc as bacc
import concourse.mybir as mybir
import concourse.tile as tile
from concourse.bass_utils import run_bass_kernel_spmd

N_CORES = 8
IMGS = 16
IMG_PER_CORE = IMGS // N_CORES  # 2
C = 256
O = 256
H = W = 64
NK = C // 128  # contraction splits
NM = O // 128  # output-channel splits
XR, XC = H + 3, W + 4  # padded input rows/cols (67, 68)
PW = W + 2  # stored plane width (66)

# (plane, rows, [(du, dv, wi, wj), ...]) in kernel iteration order.
# E-class planes have H+1 rows, O-class (row-shifted storage) H+2.
PLANES = [
    ("Ee", H + 1, [(0, 0, 0, 0), (0, 1, 0, 2), (1, 0, 2, 0), (1, 1, 2, 2)]),
    ("Eo", H + 1, [(0, 0, 0, 1), (1, 0, 2, 1)]),
    ("Oe", H + 2, [(0, 0, 1, 0), (0, 1, 1, 2)]),
    ("Oo", H + 2, [(0, 0, 1, 1)]),
]


def _chunks(rows):
    """7-row PSUM chunks paired into bands of (up to) 14 rows."""
    starts = list(range(0, rows, 7))
    ch = [(s, min(7, rows - s)) for s in starts]
    bands = [ch[i : i + 2] for i in range(0, len(ch), 2)]
    return bands


def _stat_order():
    """Stationary weight order: (m, plane_idx, tap_idx, k) -> flat index."""
    order = []
    for m in range(NM):
        for pi, (_, _, taps) in enumerate(PLANES):
            for ti in range(len(taps)):
                for k in range(NK):
                    order.append((m, pi, ti, k))
    return {key: i for i, key in enumerate(order)}


STAT_IDX = _stat_order()
NSTAT = len(STAT_IDX)  # 36
NSTAT_TOT = NSTAT + 2  # + identity, 3*identity for FIR combine matmuls

_compiled = None
LAST_RESULTS = None


def _build():
    nc = bacc.Bacc(None, target_bir_lowering=False, debug=False)
    dt = mybir.dt
    f32r, f32, bf16 = dt.float32r, dt.float32, dt.bfloat16
    MULT, ADD = mybir.AluOpType.mult, mybir.AluOpType.add

    xp_d = nc.dram_tensor(
        "xp", (IMG_PER_CORE, NK, 128, XR * XC), bf16, kind="ExternalInput"
    )
    wt_d = nc.dram_tensor("wt", (128, NSTAT_TOT * 128), bf16, kind="ExternalInput")
    b_d = nc.dram_tensor("bias", (128, NM), f32, kind="ExternalInput")
    out_d = nc.dram_tensor(
        "out", (IMG_PER_CORE, O, 2 * H, 2 * W), f32, kind="ExternalOutput"
    )

    with tile.TileContext(nc) as tc:
        with (
            tc.tile_pool(name="xpool", bufs=1) as xpool,
            tc.tile_pool(name="wpool", bufs=1) as wpool,
            tc.tile_pool(name="ybpool", bufs=4) as ybpool,
            tc.tile_pool(name="pqpool", bufs=2) as pqpool,
            tc.tile_pool(name="hpool", bufs=2) as hpool,
            tc.tile_pool(name="cpool", bufs=2) as cpool,
            tc.tile_pool(name="qpool", bufs=4) as qpool,
            tc.tile_pool(name="opool", bufs=2) as opool,
            tc.tile_pool(name="psum", bufs=6, space="PSUM") as psum_pool,
            tc.tile_pool(name="qpsum", bufs=2, space="PSUM") as qpsum,
        ):
            wt_t = wpool.tile([128, NSTAT_TOT * 128], bf16, tag="wt")
            b_t = wpool.tile([128, NM], f32, tag="bias")
            xp_t = {}

            def load_xp(img, k, split=False):
                t = xpool.tile([128, XR, XC], bf16, tag=f"xp{img}{k}")
                src = xp_d.ap()[img, k].rearrange("p (h w) -> p h w", h=XR)
                if split:
                    nc.sync.dma_start(t[:, :20, :], src[:, :20, :])
                    nc.sync.dma_start(t[:, 20:, :], src[:, 20:, :])
                else:
                    nc.sync.dma_start(t[:], src)
                xp_t[img, k] = t

            # Minimal working set first: weights for (m0, plane Ee), first xp
            # rows, then the rest.
            nc.sync.dma_start(wt_t[:, : 8 * 128], wt_d.ap()[:, : 8 * 128])
            load_xp(0, 0, split=True)
            nc.sync.dma_start(b_t[:], b_d.ap()[:])
            load_xp(0, 1, split=True)
            nc.sync.dma_start(wt_t[:, 8 * 128 :], wt_d.ap()[:, 8 * 128 :])
            load_xp(1, 0)
            load_xp(1, 1)

            I1 = wt_t[:, NSTAT * 128 : (NSTAT + 1) * 128]
            I3 = wt_t[:, (NSTAT + 1) * 128 : (NSTAT + 2) * 128]

            for img in range(IMG_PER_CORE):
                for m in range(NM):
                    # ---- stage A+B: matmul parity planes, evac, col FIR ----
                    h_t = {}
                    for name, rows, _ in PLANES:
                        h_t[name] = hpool.tile(
                            [128, rows, W], bf16, tag=f"h{name}", name=f"h{name}"
                        )

                    for cls, rows in (("E", H + 1), ("O", H + 2)):
                        pe_i, po_i = (0, 1) if cls == "E" else (2, 3)
                        _, _, pe_taps = PLANES[pe_i]
                        _, _, po_taps = PLANES[po_i]
                        for band in _chunks(rows):
                            r0 = band[0][0]
                            nr = sum(n for _, n in band)
                            psums = {}
                            for pi, taps in ((pe_i, pe_taps), (po_i, po_taps)):
                                pts = [
                                    psum_pool.tile(
                                        [128, n, PW], f32, tag="ps", name="ps"
                                    )
                                    for _, n in band
                                ]
                                n_ops = len(taps) * NK
                                acc = 0
                                for ti, (du, dv, _, _) in enumerate(taps):
                                    for k in range(NK):
                                        si = STAT_IDX[(m, pi, ti, k)]
                                        lhsT = wt_t[:, si * 128 : (si + 1) * 128]
                                        for ci, (cs, cn) in enumerate(band):
                                            rhs = xp_t[img, k][
                                                :,
                                                cs + du : cs + du + cn,
                                                dv : dv + PW,
                                            ]
                                            nc.tensor.matmul(
                                                pts[ci][:],
                                                lhsT,
                                                rhs,
                                                start=(acc == 0),
                                                stop=(acc == n_ops - 1),
                                            )
                                        acc += 1
                                psums[pi] = pts

                            # evac: n copies only (ScalarE, PSUM->SBUF bf16)
                            yb = {}
                            for pi in (pe_i, po_i):
                                n_t = ybpool.tile([128, nr, PW], bf16, tag="ybn")
                                ro = 0
                                for ci, (cs, cn) in enumerate(band):
                                    nc.scalar.copy(
                                        n_t[:, ro : ro + cn, :], psums[pi][ci][:]
                                    )
                                    ro += cn
                                yb[pi] = n_t

                            ne = yb[pe_i]
                            no = yb[po_i]
                            # col FIR (unnormalized x4), all on DVE:
                            #   he = 3*ne[0] + ne[1] + 3*no[1] + no[0]
                            #      = (3*ne[0]+no[0]) + (3*no[1]+ne[1])
                            #   ho = 3*ne[1] + ne[0] + 3*no[1] + no[2]
                            #      = 3*(ne[1]+no[1]) + (ne[0]+no[2])
                            he = h_t["Ee" if cls == "E" else "Oe"]
                            ho = h_t["Eo" if cls == "E" else "Oo"]
                            # STT is 1x-only on DVE; route scales through
                            # tensor_scalar (4x) and keep TTs 2x where the
                            # operand byte-alignment allows (even col offsets).
                            ne3 = pqpool.tile([128, nr, PW], bf16, tag="n3")
                            A_ = pqpool.tile([128, nr, W], bf16, tag="Ac")
                            B_ = pqpool.tile([128, nr, W], bf16, tag="Bc")
                            U_ = pqpool.tile([128, nr, W], bf16, tag="Uc")
                            T_ = pqpool.tile([128, nr, W], bf16, tag="Tc")
                            T3 = pqpool.tile([128, nr, W], bf16, tag="T3")
                            nc.vector.tensor_scalar_mul(ne3[:], ne[:], 3.0)
                            nc.vector.tensor_tensor(
                                A_[:], ne3[:, :, 0:W], no[:, :, 0:W], ADD
                            )
                            nc.vector.scalar_tensor_tensor(
                                B_[:], no[:, :, 1 : W + 1], 3.0,
                                ne[:, :, 1 : W + 1], MULT, ADD
                            )
                            nc.vector.tensor_tensor(
                                he[:, r0 : r0 + nr, :], A_[:], B_[:], ADD
                            )
                            nc.vector.tensor_tensor(
                                U_[:], ne[:, :, 0:W], no[:, :, 2 : W + 2], ADD
                            )
                            nc.vector.tensor_tensor(
                                T_[:], ne[:, :, 1 : W + 1], no[:, :, 1 : W + 1], ADD
                            )
                            nc.vector.tensor_scalar_mul(T3[:], T_[:], 3.0)
                            nc.vector.tensor_tensor(
                                ho[:, r0 : r0 + nr, :], T3[:], U_[:], ADD
                            )

                    # ---- stage C: row FIR + final ----
                    # even out-rows (alpha=0) on DVE, odd (alpha=1) on TensorE
                    for qb in range(4):
                        u0 = qb * 16
                        out_sb = opool.tile([128, 32, 2 * W], f32)
                        for cj, cp in enumerate(("e", "o")):
                            A = h_t["E" + cp]
                            B = h_t["O" + cp]
                            # oE = (3A[u] + A[u+1]) + (3B[u+1] + B[u])
                            A3 = cpool.tile([128, 16, W], bf16, tag="A3")
                            B3 = cpool.tile([128, 16, W], bf16, tag="B3")
                            P_ = cpool.tile([128, 16, W], bf16, tag="Pr")
                            Q_ = cpool.tile([128, 16, W], bf16, tag="Qr")
                            nc.vector.tensor_scalar_mul(
                                A3[:], A[:, u0 : u0 + 16, :], 3.0
                            )
                            nc.vector.tensor_scalar_mul(
                                B3[:], B[:, u0 + 1 : u0 + 17, :], 3.0
                            )
                            nc.vector.tensor_tensor(
                                P_[:], A3[:], A[:, u0 + 1 : u0 + 17, :], ADD
                            )
                            nc.vector.tensor_tensor(
                                Q_[:], B3[:], B[:, u0 : u0 + 16, :], ADD
                            )
                            oE = qpool.tile([128, 16, W], bf16, tag="oE")
                            nc.vector.tensor_tensor(oE[:], P_[:], Q_[:], ADD)
                            nc.scalar.activation(
                                out_sb[:, 0::2, cj::2],
                                oE[:],
                                mybir.ActivationFunctionType.Identity,
                                bias=b_t[:, m : m + 1],
                                scale=1.0 / 16.0,
                            )
                            # oO = A[u] + 3A[u+1] + 3B[u+1] + B[u+2]  (PE psum)
                            for sub in range(2):
                                u1 = u0 + sub * 8
                                qp = qpsum.tile([128, 8, W], f32, tag="qp",
                                                name="qp")
                                nc.tensor.matmul(
                                    qp[:], I1, A[:, u1 : u1 + 8, :],
                                    start=True, stop=False,
                                )
                                nc.tensor.matmul(
                                    qp[:], I3, A[:, u1 + 1 : u1 + 9, :],
                                    start=False, stop=False,
                                )
                                nc.tensor.matmul(
                                    qp[:], I3, B[:, u1 + 1 : u1 + 9, :],
                                    start=False, stop=False,
                                )
                                nc.tensor.matmul(
                                    qp[:], I1, B[:, u1 + 2 : u1 + 10, :],
                                    start=False, stop=True,
                                )
                                nc.scalar.activation(
                                    out_sb[:, sub * 16 + 1 : sub * 16 + 16 : 2,
                                           cj::2],
                                    qp[:],
                                    mybir.ActivationFunctionType.Identity,
                                    bias=b_t[:, m : m + 1],
                                    scale=1.0 / 16.0,
                                )
                        nc.sync.dma_start(
                            out_d.ap()[
                                img,
                                m * 128 : (m + 1) * 128,
                                qb * 32 : (qb + 1) * 32,
                                :,
                            ],
                            out_sb[:],
                        )

    nc.compile()
    return nc


def _prep_weights(w):
    """w (256,256,3,3) -> [c_local, (stat idx, o_local)] fp32."""
    wt = np.empty((128, NSTAT_TOT, 128), dtype=np.float32)
    wt[:, NSTAT, :] = np.eye(128, dtype=np.float32)
    wt[:, NSTAT + 1, :] = 3.0 * np.eye(128, dtype=np.float32)
    for m in range(NM):
        for pi, (_, _, taps) in enumerate(PLANES):
            for ti, (_, _, wi, wj) in enumerate(taps):
                for k in range(NK):
                    si = STAT_IDX[(m, pi, ti, k)]
                    sub = w[m * 128 : (m + 1) * 128, k * 128 : (k + 1) * 128, wi, wj]
                    wt[:, si, :] = sub.T
    return np.ascontiguousarray(wt.reshape(128, NSTAT_TOT * 128))


def kernel(x, w, b):
    global _compiled, LAST_RESULTS
    if _compiled is None:
        _compiled = _build()
    nc = _compiled

    x = np.asarray(x, dtype=np.float32)
    w = np.asarray(w, dtype=np.float32)
    b = np.asarray(b, dtype=np.float32)

    import ml_dtypes

    wt = _prep_weights(w).astype(ml_dtypes.bfloat16)
    b2 = np.ascontiguousarray(b.reshape(NM, 128).T)
    xp = np.zeros((IMGS, C, XR, XC), dtype=np.float32)
    xp[:, :, 1 : H + 1, 1 : W + 1] = x
    xp = np.ascontiguousarray(
        xp.reshape(N_CORES, IMG_PER_CORE, NK, 128, XR * XC)
    ).astype(ml_dtypes.bfloat16)

    in_maps = [
        {"xp": xp[core], "wt": wt, "bias": b2} for core in range(N_CORES)
    ]
    try:
        res = run_bass_kernel_spmd(nc, in_maps, list(range(N_CORES)))
    except ModuleNotFoundError:
        import os

        os.environ["BASS_NEVER_TRACE"] = "1"
        res = run_bass_kernel_spmd(nc, in_maps, list(range(N_CORES)))
    LAST_RESULTS = res
    out = np.concatenate([res.results[i]["out"] for i in range(N_CORES)], axis=0)
    return out



# revision 10
# speedup vs baseline: 1.4570x; 1.0184x over previous
"""StyleGAN2 fused upsample2x + 3x3 conv + FIR(1,3,3,1) + bias — TRN2 Bass kernel v2.

Unlike v1 (which folded the FIR into the conv weights, 4x the matmul work),
this version computes the four parity planes of the stride-2 transposed conv
directly (9 taps total across planes -> 4x fewer MACs on TensorE), then applies
the separable FIR (1,3,3,1)/4 per dimension as fused scalar_tensor_tensor ops:

  y parity planes (PSUM, fp32) --ScalarE copy--> bf16 SBUF (plus shifted-by-1
  copies via GPSIMD so every DVE operand stays 4B-aligned => 2x bf16 mode)
  --DVE col FIR--> h planes --DVE row FIR--> quadrants
  --ScalarE scale(1/16)+bias--> interleaved fp32 out --DMA--> HBM.

Data-parallel over batch: 2 images per core, 8 cores.  Matmuls in float32r.
"""

import sys

sys.path.insert(0, "/opt/trn_rl_repo")

import numpy as np

import concourse.bacc as bacc
import concourse.mybir as mybir
import concourse.tile as tile
from concourse.bass_utils import run_bass_kernel_spmd

N_CORES = 8
IMGS = 16
IMG_PER_CORE = IMGS // N_CORES  # 2
C = 256
O = 256
H = W = 64
NK = C // 128  # contraction splits
NM = O // 128  # output-channel splits
XR, XC = H + 3, W + 4  # padded input rows/cols (67, 68)
PW = W + 2  # stored plane width (66)

# (plane, rows, [(du, dv, wi, wj), ...]) in kernel iteration order.
# E-class planes have H+1 rows, O-class (row-shifted storage) H+2.
PLANES = [
    ("Ee", H + 1, [(0, 0, 0, 0), (0, 1, 0, 2), (1, 0, 2, 0), (1, 1, 2, 2)]),
    ("Eo", H + 1, [(0, 0, 0, 1), (1, 0, 2, 1)]),
    ("Oe", H + 2, [(0, 0, 1, 0), (0, 1, 1, 2)]),
    ("Oo", H + 2, [(0, 0, 1, 1)]),
]


def _chunks(rows):
    """7-row PSUM chunks paired into bands of (up to) 14 rows."""
    starts = list(range(0, rows, 7))
    ch = [(s, min(7, rows - s)) for s in starts]
    bands = [ch[i : i + 2] for i in range(0, len(ch), 2)]
    return bands


def _stat_order():
    """Stationary weight order: (m, plane_idx, tap_idx, k) -> flat index."""
    order = []
    for m in range(NM):
        for pi, (_, _, taps) in enumerate(PLANES):
            for ti in range(len(taps)):
                for k in range(NK):
                    order.append((m, pi, ti, k))
    return {key: i for i, key in enumerate(order)}


STAT_IDX = _stat_order()
NSTAT = len(STAT_IDX)  # 36
NSTAT_TOT = NSTAT + 2  # + identity, 3*identity for FIR combine matmuls

_compiled = None
LAST_RESULTS = None


def _build():
    nc = bacc.Bacc(None, target_bir_lowering=False, debug=False)
    dt = mybir.dt
    f32r, f32, bf16 = dt.float32r, dt.float32, dt.bfloat16
    MULT, ADD = mybir.AluOpType.mult, mybir.AluOpType.add

    xp_d = nc.dram_tensor(
        "xp", (IMG_PER_CORE, NK, 128, XR * XC), bf16, kind="ExternalInput"
    )
    wt_d = nc.dram_tensor("wt", (128, NSTAT_TOT * 128), bf16, kind="ExternalInput")
    b_d = nc.dram_tensor("bias", (128, NM), f32, kind="ExternalInput")
    out_d = nc.dram_tensor(
        "out", (IMG_PER_CORE, O, 2 * H, 2 * W), f32, kind="ExternalOutput"
    )

    with tile.TileContext(nc) as tc:
        with (
            tc.tile_pool(name="xpool", bufs=1) as xpool,
            tc.tile_pool(name="wpool", bufs=1) as wpool,
            tc.tile_pool(name="ybpool", bufs=4) as ybpool,
            tc.tile_pool(name="pqpool", bufs=2) as pqpool,
            tc.tile_pool(name="hpool", bufs=2) as hpool,
            tc.tile_pool(name="cpool", bufs=2) as cpool,
            tc.tile_pool(name="qpool", bufs=4) as qpool,
            tc.tile_pool(name="opool", bufs=2) as opool,
            tc.tile_pool(name="psum", bufs=6, space="PSUM") as psum_pool,
            tc.tile_pool(name="qpsum", bufs=2, space="PSUM") as qpsum,
        ):
            wt_t = wpool.tile([128, NSTAT_TOT * 128], bf16, tag="wt")
            b_t = wpool.tile([128, NM], f32, tag="bias")
            xp_t = {}

            def load_xp(img, k, split=False):
                t = xpool.tile([128, XR, XC], bf16, tag=f"xp{img}{k}")
                src = xp_d.ap()[img, k].rearrange("p (h w) -> p h w", h=XR)
                if split:
                    nc.sync.dma_start(t[:, :20, :], src[:, :20, :])
                    nc.sync.dma_start(t[:, 20:, :], src[:, 20:, :])
                else:
                    nc.sync.dma_start(t[:], src)
                xp_t[img, k] = t

            # Minimal working set first: weights for (m0, plane Ee), first xp
            # rows, then the rest.
            nc.sync.dma_start(wt_t[:, : 8 * 128], wt_d.ap()[:, : 8 * 128])
            load_xp(0, 0, split=True)
            nc.sync.dma_start(b_t[:], b_d.ap()[:])
            load_xp(0, 1, split=True)
            nc.sync.dma_start(wt_t[:, 8 * 128 :], wt_d.ap()[:, 8 * 128 :])
            load_xp(1, 0)
            load_xp(1, 1)

            I1 = wt_t[:, NSTAT * 128 : (NSTAT + 1) * 128]
            I3 = wt_t[:, (NSTAT + 1) * 128 : (NSTAT + 2) * 128]

            # Software-pipelined over (img, m) units: stage C of unit u-1 is
            # emitted after stage A+B of unit u so the PE/Scalar FIFOs never
            # stall on the current unit's col-FIR completing.
            units = [
                (img, m) for img in range(IMG_PER_CORE) for m in range(NM)
            ]
            done_h = []

            def stage_ab(img, m):
                    # ---- stage A+B: matmul parity planes, evac, col FIR ----
                    h_t = {}
                    for name, rows, _ in PLANES:
                        h_t[name] = hpool.tile(
                            [128, rows, W], bf16, tag=f"h{name}", name=f"h{name}"
                        )

                    for cls, rows in (("E", H + 1), ("O", H + 2)):
                        pe_i, po_i = (0, 1) if cls == "E" else (2, 3)
                        _, _, pe_taps = PLANES[pe_i]
                        _, _, po_taps = PLANES[po_i]
                        for band in _chunks(rows):
                            r0 = band[0][0]
                            nr = sum(n for _, n in band)
                            psums = {}
                            for pi, taps in ((pe_i, pe_taps), (po_i, po_taps)):
                                pts = [
                                    psum_pool.tile(
                                        [128, n, PW], f32, tag="ps", name="ps"
                                    )
                                    for _, n in band
                                ]
                                n_ops = len(taps) * NK
                                acc = 0
                                for ti, (du, dv, _, _) in enumerate(taps):
                                    for k in range(NK):
                                        si = STAT_IDX[(m, pi, ti, k)]
                                        lhsT = wt_t[:, si * 128 : (si + 1) * 128]
                                        for ci, (cs, cn) in enumerate(band):
                                            rhs = xp_t[img, k][
                                                :,
                                                cs + du : cs + du + cn,
                                                dv : dv + PW,
                                            ]
                                            nc.tensor.matmul(
                                                pts[ci][:],
                                                lhsT,
                                                rhs,
                                                start=(acc == 0),
                                                stop=(acc == n_ops - 1),
                                            )
                                        acc += 1
                                psums[pi] = pts

                            # evac: n copies only (ScalarE, PSUM->SBUF bf16)
                            yb = {}
                            for pi in (pe_i, po_i):
                                n_t = ybpool.tile([128, nr, PW], bf16, tag="ybn")
                                ro = 0
                                for ci, (cs, cn) in enumerate(band):
                                    nc.scalar.copy(
                                        n_t[:, ro : ro + cn, :], psums[pi][ci][:]
                                    )
                                    ro += cn
                                yb[pi] = n_t

                            ne = yb[pe_i]
                            no = yb[po_i]
                            # col FIR (unnormalized x4), all on DVE:
                            #   he = 3*ne[0] + ne[1] + 3*no[1] + no[0]
                            #      = (3*ne[0]+no[0]) + (3*no[1]+ne[1])
                            #   ho = 3*ne[1] + ne[0] + 3*no[1] + no[2]
                            #      = 3*(ne[1]+no[1]) + (ne[0]+no[2])
                            he = h_t["Ee" if cls == "E" else "Oe"]
                            ho = h_t["Eo" if cls == "E" else "Oo"]
                            # STT is 1x-only on DVE; route scales through
                            # tensor_scalar (4x) and keep TTs 2x where the
                            # operand byte-alignment allows (even col offsets).
                            ne3 = pqpool.tile([128, nr, PW], bf16, tag="n3")
                            A_ = pqpool.tile([128, nr, W], bf16, tag="Ac")
                            B_ = pqpool.tile([128, nr, W], bf16, tag="Bc")
                            U_ = pqpool.tile([128, nr, W], bf16, tag="Uc")
                            T_ = pqpool.tile([128, nr, W], bf16, tag="Tc")
                            T3 = pqpool.tile([128, nr, W], bf16, tag="T3")
                            nc.vector.tensor_scalar_mul(ne3[:], ne[:], 3.0)
                            nc.vector.tensor_tensor(
                                A_[:], ne3[:, :, 0:W], no[:, :, 0:W], ADD
                            )
                            nc.vector.scalar_tensor_tensor(
                                B_[:], no[:, :, 1 : W + 1], 3.0,
                                ne[:, :, 1 : W + 1], MULT, ADD
                            )
                            nc.vector.tensor_tensor(
                                he[:, r0 : r0 + nr, :], A_[:], B_[:], ADD
                            )
                            nc.vector.tensor_tensor(
                                U_[:], ne[:, :, 0:W], no[:, :, 2 : W + 2], ADD
                            )
                            nc.vector.tensor_tensor(
                                T_[:], ne[:, :, 1 : W + 1], no[:, :, 1 : W + 1], ADD
                            )
                            nc.vector.tensor_scalar_mul(T3[:], T_[:], 3.0)
                            nc.vector.tensor_tensor(
                                ho[:, r0 : r0 + nr, :], T3[:], U_[:], ADD
                            )
                    return h_t

            def stage_c(h_t, img, m):
                    # ---- stage C: row FIR + final ----
                    # even out-rows (alpha=0) on DVE, odd (alpha=1) on TensorE
                    for qb in range(4):
                        u0 = qb * 16
                        out_sb = opool.tile([128, 32, 2 * W], f32)
                        for cj, cp in enumerate(("e", "o")):
                            A = h_t["E" + cp]
                            B = h_t["O" + cp]
                            # oE = (3A[u] + A[u+1]) + (3B[u+1] + B[u])
                            A3 = cpool.tile([128, 16, W], bf16, tag="A3")
                            B3 = cpool.tile([128, 16, W], bf16, tag="B3")
                            P_ = cpool.tile([128, 16, W], bf16, tag="Pr")
                            Q_ = cpool.tile([128, 16, W], bf16, tag="Qr")
                            nc.vector.tensor_scalar_mul(
                                A3[:], A[:, u0 : u0 + 16, :], 3.0
                            )
                            nc.vector.tensor_scalar_mul(
                                B3[:], B[:, u0 + 1 : u0 + 17, :], 3.0
                            )
                            nc.vector.tensor_tensor(
                                P_[:], A3[:], A[:, u0 + 1 : u0 + 17, :], ADD
                            )
                            nc.vector.tensor_tensor(
                                Q_[:], B3[:], B[:, u0 : u0 + 16, :], ADD
                            )
                            oE = qpool.tile([128, 16, W], bf16, tag="oE")
                            nc.vector.tensor_tensor(oE[:], P_[:], Q_[:], ADD)
                            nc.scalar.activation(
                                out_sb[:, 0::2, cj::2],
                                oE[:],
                                mybir.ActivationFunctionType.Identity,
                                bias=b_t[:, m : m + 1],
                                scale=1.0 / 16.0,
                            )
                            # oO = A[u] + 3A[u+1] + 3B[u+1] + B[u+2]  (PE psum)
                            for sub in range(2):
                                u1 = u0 + sub * 8
                                qp = qpsum.tile([128, 8, W], f32, tag="qp",
                                                name="qp")
                                nc.tensor.matmul(
                                    qp[:], I1, A[:, u1 : u1 + 8, :],
                                    start=True, stop=False,
                                )
                                nc.tensor.matmul(
                                    qp[:], I3, A[:, u1 + 1 : u1 + 9, :],
                                    start=False, stop=False,
                                )
                                nc.tensor.matmul(
                                    qp[:], I3, B[:, u1 + 1 : u1 + 9, :],
                                    start=False, stop=False,
                                )
                                nc.tensor.matmul(
                                    qp[:], I1, B[:, u1 + 2 : u1 + 10, :],
                                    start=False, stop=True,
                                )
                                nc.scalar.activation(
                                    out_sb[:, sub * 16 + 1 : sub * 16 + 16 : 2,
                                           cj::2],
                                    qp[:],
                                    mybir.ActivationFunctionType.Identity,
                                    bias=b_t[:, m : m + 1],
                                    scale=1.0 / 16.0,
                                )
                        nc.sync.dma_start(
                            out_d.ap()[
                                img,
                                m * 128 : (m + 1) * 128,
                                qb * 32 : (qb + 1) * 32,
                                :,
                            ],
                            out_sb[:],
                        )

            for img, m in units:
                stage_c(stage_ab(img, m), img, m)

    nc.compile()
    return nc


def _prep_weights(w):
    """w (256,256,3,3) -> [c_local, (stat idx, o_local)] fp32."""
    wt = np.empty((128, NSTAT_TOT, 128), dtype=np.float32)
    wt[:, NSTAT, :] = np.eye(128, dtype=np.float32)
    wt[:, NSTAT + 1, :] = 3.0 * np.eye(128, dtype=np.float32)
    for m in range(NM):
        for pi, (_, _, taps) in enumerate(PLANES):
            for ti, (_, _, wi, wj) in enumerate(taps):
                for k in range(NK):
                    si = STAT_IDX[(m, pi, ti, k)]
                    sub = w[m * 128 : (m + 1) * 128, k * 128 : (k + 1) * 128, wi, wj]
                    wt[:, si, :] = sub.T
    return np.ascontiguousarray(wt.reshape(128, NSTAT_TOT * 128))


def kernel(x, w, b):
    global _compiled, LAST_RESULTS
    if _compiled is None:
        _compiled = _build()
    nc = _compiled

    x = np.asarray(x, dtype=np.float32)
    w = np.asarray(w, dtype=np.float32)
    b = np.asarray(b, dtype=np.float32)

    import ml_dtypes

    wt = _prep_weights(w).astype(ml_dtypes.bfloat16)
    b2 = np.ascontiguousarray(b.reshape(NM, 128).T)
    xp = np.zeros((IMGS, C, XR, XC), dtype=np.float32)
    xp[:, :, 1 : H + 1, 1 : W + 1] = x
    xp = np.ascontiguousarray(
        xp.reshape(N_CORES, IMG_PER_CORE, NK, 128, XR * XC)
    ).astype(ml_dtypes.bfloat16)

    in_maps = [
        {"xp": xp[core], "wt": wt, "bias": b2} for core in range(N_CORES)
    ]
    try:
        res = run_bass_kernel_spmd(nc, in_maps, list(range(N_CORES)))
    except ModuleNotFoundError:
        import os

        os.environ["BASS_NEVER_TRACE"] = "1"
        res = run_bass_kernel_spmd(nc, in_maps, list(range(N_CORES)))
    LAST_RESULTS = res
    out = np.concatenate([res.results[i]["out"] for i in range(N_CORES)], axis=0)
    return out



# revision 11
# speedup vs baseline: 1.5007x; 1.0300x over previous
"""StyleGAN2 fused upsample2x + 3x3 conv + FIR(1,3,3,1) + bias — TRN2 Bass kernel v2.

Unlike v1 (which folded the FIR into the conv weights, 4x the matmul work),
this version computes the four parity planes of the stride-2 transposed conv
directly (9 taps total across planes -> 4x fewer MACs on TensorE), then applies
the separable FIR (1,3,3,1)/4 per dimension as fused scalar_tensor_tensor ops:

  y parity planes (PSUM, fp32) --ScalarE copy--> bf16 SBUF (plus shifted-by-1
  copies via GPSIMD so every DVE operand stays 4B-aligned => 2x bf16 mode)
  --DVE col FIR--> h planes --DVE row FIR--> quadrants
  --ScalarE scale(1/16)+bias--> interleaved fp32 out --DMA--> HBM.

Data-parallel over batch: 2 images per core, 8 cores.  Matmuls in float32r.
"""

import sys

sys.path.insert(0, "/opt/trn_rl_repo")

import numpy as np

import concourse.bacc as bacc
import concourse.mybir as mybir
import concourse.tile as tile
from concourse.bass_utils import run_bass_kernel_spmd

N_CORES = 8
IMGS = 16
IMG_PER_CORE = IMGS // N_CORES  # 2
C = 256
O = 256
H = W = 64
NK = C // 128  # contraction splits
NM = O // 128  # output-channel splits
XR, XC = H + 3, W + 4  # padded input rows/cols (67, 68)
PW = W + 2  # stored plane width (66)

# (plane, rows, [(du, dv, wi, wj), ...]) in kernel iteration order.
# E-class planes have H+1 rows, O-class (row-shifted storage) H+2.
PLANES = [
    ("Ee", H + 1, [(0, 0, 0, 0), (0, 1, 0, 2), (1, 0, 2, 0), (1, 1, 2, 2)]),
    ("Eo", H + 1, [(0, 0, 0, 1), (1, 0, 2, 1)]),
    ("Oe", H + 2, [(0, 0, 1, 0), (0, 1, 1, 2)]),
    ("Oo", H + 2, [(0, 0, 1, 1)]),
]


def _chunks(rows):
    """7-row PSUM chunks paired into bands of (up to) 14 rows."""
    starts = list(range(0, rows, 7))
    ch = [(s, min(7, rows - s)) for s in starts]
    bands = [ch[i : i + 2] for i in range(0, len(ch), 2)]
    return bands


def _stat_order():
    """Stationary weight order: (m, plane_idx, tap_idx, k) -> flat index."""
    order = []
    for m in range(NM):
        for pi, (_, _, taps) in enumerate(PLANES):
            for ti in range(len(taps)):
                for k in range(NK):
                    order.append((m, pi, ti, k))
    return {key: i for i, key in enumerate(order)}


STAT_IDX = _stat_order()
NSTAT = len(STAT_IDX)  # 36
NSTAT_TOT = NSTAT + 2  # + identity, 3*identity for FIR combine matmuls

_compiled = None
LAST_RESULTS = None


def _build():
    nc = bacc.Bacc(None, target_bir_lowering=False, debug=False)
    dt = mybir.dt
    f32r, f32, bf16 = dt.float32r, dt.float32, dt.bfloat16
    MULT, ADD = mybir.AluOpType.mult, mybir.AluOpType.add

    xp_d = nc.dram_tensor(
        "xp", (IMG_PER_CORE, NK, 128, XR * XC), bf16, kind="ExternalInput"
    )
    wt_d = nc.dram_tensor("wt", (128, NSTAT_TOT * 128), bf16, kind="ExternalInput")
    b_d = nc.dram_tensor("bias", (128, NM), f32, kind="ExternalInput")
    out_d = nc.dram_tensor(
        "out", (IMG_PER_CORE, O, 2 * H, 2 * W), f32, kind="ExternalOutput"
    )

    with tile.TileContext(nc) as tc:
        with (
            tc.tile_pool(name="xpool", bufs=1) as xpool,
            tc.tile_pool(name="wpool", bufs=1) as wpool,
            tc.tile_pool(name="ybpool", bufs=4) as ybpool,
            tc.tile_pool(name="pqpool", bufs=2) as pqpool,
            tc.tile_pool(name="hpool", bufs=2) as hpool,
            tc.tile_pool(name="cpool", bufs=2) as cpool,
            tc.tile_pool(name="qpool", bufs=4) as qpool,
            tc.tile_pool(name="opool", bufs=2) as opool,
            tc.tile_pool(name="psum", bufs=6, space="PSUM") as psum_pool,
            tc.tile_pool(name="qpsum", bufs=2, space="PSUM") as qpsum,
        ):
            wt_t = wpool.tile([128, NSTAT_TOT * 128], bf16, tag="wt")
            b_t = wpool.tile([128, NM], f32, tag="bias")
            xp_t = {}

            def load_xp(img, k, split=False):
                t = xpool.tile([128, XR, XC], bf16, tag=f"xp{img}{k}")
                src = xp_d.ap()[img, k].rearrange("p (h w) -> p h w", h=XR)
                if split:
                    nc.sync.dma_start(t[:, :20, :], src[:, :20, :])
                    nc.sync.dma_start(t[:, 20:, :], src[:, 20:, :])
                else:
                    nc.sync.dma_start(t[:], src)
                xp_t[img, k] = t

            # Minimal working set first: weights for (m0, plane Ee), first xp
            # rows, then the rest.
            nc.sync.dma_start(wt_t[:, : 8 * 128], wt_d.ap()[:, : 8 * 128])
            load_xp(0, 0, split=True)
            nc.sync.dma_start(b_t[:], b_d.ap()[:])
            load_xp(0, 1, split=True)
            nc.sync.dma_start(wt_t[:, 8 * 128 :], wt_d.ap()[:, 8 * 128 :])
            load_xp(1, 0)
            load_xp(1, 1)

            I1 = wt_t[:, NSTAT * 128 : (NSTAT + 1) * 128]
            I3 = wt_t[:, (NSTAT + 1) * 128 : (NSTAT + 2) * 128]

            # Software-pipelined over (img, m) units: stage C of unit u-1 is
            # emitted after stage A+B of unit u so the PE/Scalar FIFOs never
            # stall on the current unit's col-FIR completing.
            units = [
                (img, m) for img in range(IMG_PER_CORE) for m in range(NM)
            ]
            done_h = []

            def stage_ab(img, m):
                    # ---- stage A+B: matmul parity planes, evac, col FIR ----
                    h_t = {}
                    for name, rows, _ in PLANES:
                        h_t[name] = hpool.tile(
                            [128, rows, W], bf16, tag=f"h{name}", name=f"h{name}"
                        )

                    for cls, rows in (("E", H + 1), ("O", H + 2)):
                        pe_i, po_i = (0, 1) if cls == "E" else (2, 3)
                        _, _, pe_taps = PLANES[pe_i]
                        _, _, po_taps = PLANES[po_i]
                        for band in _chunks(rows):
                            r0 = band[0][0]
                            nr = sum(n for _, n in band)
                            psums = {}
                            for pi, taps in ((pe_i, pe_taps), (po_i, po_taps)):
                                pts = [
                                    psum_pool.tile(
                                        [128, n, PW], f32, tag="ps", name="ps"
                                    )
                                    for _, n in band
                                ]
                                n_ops = len(taps) * NK
                                acc = 0
                                for ti, (du, dv, _, _) in enumerate(taps):
                                    for k in range(NK):
                                        si = STAT_IDX[(m, pi, ti, k)]
                                        lhsT = wt_t[:, si * 128 : (si + 1) * 128]
                                        for ci, (cs, cn) in enumerate(band):
                                            rhs = xp_t[img, k][
                                                :,
                                                cs + du : cs + du + cn,
                                                dv : dv + PW,
                                            ]
                                            nc.tensor.matmul(
                                                pts[ci][:],
                                                lhsT,
                                                rhs,
                                                start=(acc == 0),
                                                stop=(acc == n_ops - 1),
                                            )
                                        acc += 1
                                psums[pi] = pts

                            # evac: n copies only (ScalarE, PSUM->SBUF bf16)
                            yb = {}
                            for pi in (pe_i, po_i):
                                n_t = ybpool.tile([128, nr, PW], bf16, tag="ybn")
                                ro = 0
                                for ci, (cs, cn) in enumerate(band):
                                    nc.scalar.copy(
                                        n_t[:, ro : ro + cn, :], psums[pi][ci][:]
                                    )
                                    ro += cn
                                yb[pi] = n_t

                            ne = yb[pe_i]
                            no = yb[po_i]
                            # col FIR (unnormalized x4), all on DVE:
                            #   he = 3*ne[0] + ne[1] + 3*no[1] + no[0]
                            #      = (3*ne[0]+no[0]) + (3*no[1]+ne[1])
                            #   ho = 3*ne[1] + ne[0] + 3*no[1] + no[2]
                            #      = 3*(ne[1]+no[1]) + (ne[0]+no[2])
                            he = h_t["Ee" if cls == "E" else "Oe"]
                            ho = h_t["Eo" if cls == "E" else "Oo"]
                            if cls == "E" and r0 == 28:
                                # offload one band per unit to TensorE: the
                                # FIR combine as identity-matmul accumulation
                                combos = (
                                    (he, ((I3, ne, 0), (I1, ne, 1),
                                          (I3, no, 1), (I1, no, 0))),
                                    (ho, ((I1, ne, 0), (I3, ne, 1),
                                          (I3, no, 1), (I1, no, 2))),
                                )
                                for dst, taps4 in combos:
                                    ro = 0
                                    for ci, (cs, cn) in enumerate(band):
                                        hp = qpsum.tile(
                                            [128, 8, W], f32, tag="qp",
                                            name="qp",
                                        )
                                        for j, (ww, src, dv) in enumerate(
                                            taps4
                                        ):
                                            nc.tensor.matmul(
                                                hp[:, 0:cn, :],
                                                ww,
                                                src[:, ro : ro + cn,
                                                    dv : dv + W],
                                                start=(j == 0),
                                                stop=(j == 3),
                                            )
                                        nc.scalar.copy(
                                            dst[:, r0 + ro : r0 + ro + cn, :],
                                            hp[:, 0:cn, :],
                                        )
                                        ro += cn
                                continue
                            # STT is 1x-only on DVE; route scales through
                            # tensor_scalar (4x) and keep TTs 2x where the
                            # operand byte-alignment allows (even col offsets).
                            ne3 = pqpool.tile([128, nr, PW], bf16, tag="n3")
                            A_ = pqpool.tile([128, nr, W], bf16, tag="Ac")
                            B_ = pqpool.tile([128, nr, W], bf16, tag="Bc")
                            U_ = pqpool.tile([128, nr, W], bf16, tag="Uc")
                            T_ = pqpool.tile([128, nr, W], bf16, tag="Tc")
                            T3 = pqpool.tile([128, nr, W], bf16, tag="T3")
                            nc.vector.tensor_scalar_mul(ne3[:], ne[:], 3.0)
                            nc.vector.tensor_tensor(
                                A_[:], ne3[:, :, 0:W], no[:, :, 0:W], ADD
                            )
                            nc.vector.scalar_tensor_tensor(
                                B_[:], no[:, :, 1 : W + 1], 3.0,
                                ne[:, :, 1 : W + 1], MULT, ADD
                            )
                            nc.vector.tensor_tensor(
                                he[:, r0 : r0 + nr, :], A_[:], B_[:], ADD
                            )
                            nc.vector.tensor_tensor(
                                U_[:], ne[:, :, 0:W], no[:, :, 2 : W + 2], ADD
                            )
                            nc.vector.tensor_tensor(
                                T_[:], ne[:, :, 1 : W + 1], no[:, :, 1 : W + 1], ADD
                            )
                            nc.vector.tensor_scalar_mul(T3[:], T_[:], 3.0)
                            nc.vector.tensor_tensor(
                                ho[:, r0 : r0 + nr, :], T3[:], U_[:], ADD
                            )
                    return h_t

            def stage_c(h_t, img, m):
                    # ---- stage C: row FIR + final ----
                    # even out-rows (alpha=0) on DVE, odd (alpha=1) on TensorE
                    for qb in range(4):
                        u0 = qb * 16
                        out_sb = opool.tile([128, 32, 2 * W], f32)
                        for cj, cp in enumerate(("e", "o")):
                            A = h_t["E" + cp]
                            B = h_t["O" + cp]
                            # oE = (3A[u] + A[u+1]) + (3B[u+1] + B[u])
                            A3 = cpool.tile([128, 16, W], bf16, tag="A3")
                            B3 = cpool.tile([128, 16, W], bf16, tag="B3")
                            P_ = cpool.tile([128, 16, W], bf16, tag="Pr")
                            Q_ = cpool.tile([128, 16, W], bf16, tag="Qr")
                            nc.vector.tensor_scalar_mul(
                                A3[:], A[:, u0 : u0 + 16, :], 3.0
                            )
                            nc.vector.tensor_scalar_mul(
                                B3[:], B[:, u0 + 1 : u0 + 17, :], 3.0
                            )
                            nc.vector.tensor_tensor(
                                P_[:], A3[:], A[:, u0 + 1 : u0 + 17, :], ADD
                            )
                            nc.vector.tensor_tensor(
                                Q_[:], B3[:], B[:, u0 : u0 + 16, :], ADD
                            )
                            oE = qpool.tile([128, 16, W], bf16, tag="oE")
                            nc.vector.tensor_tensor(oE[:], P_[:], Q_[:], ADD)
                            nc.scalar.activation(
                                out_sb[:, 0::2, cj::2],
                                oE[:],
                                mybir.ActivationFunctionType.Identity,
                                bias=b_t[:, m : m + 1],
                                scale=1.0 / 16.0,
                            )
                            # oO = A[u] + 3A[u+1] + 3B[u+1] + B[u+2]  (PE psum)
                            for sub in range(2):
                                u1 = u0 + sub * 8
                                qp = qpsum.tile([128, 8, W], f32, tag="qp",
                                                name="qp")
                                nc.tensor.matmul(
                                    qp[:], I1, A[:, u1 : u1 + 8, :],
                                    start=True, stop=False,
                                )
                                nc.tensor.matmul(
                                    qp[:], I3, A[:, u1 + 1 : u1 + 9, :],
                                    start=False, stop=False,
                                )
                                nc.tensor.matmul(
                                    qp[:], I3, B[:, u1 + 1 : u1 + 9, :],
                                    start=False, stop=False,
                                )
                                nc.tensor.matmul(
                                    qp[:], I1, B[:, u1 + 2 : u1 + 10, :],
                                    start=False, stop=True,
                                )
                                nc.scalar.activation(
                                    out_sb[:, sub * 16 + 1 : sub * 16 + 16 : 2,
                                           cj::2],
                                    qp[:],
                                    mybir.ActivationFunctionType.Identity,
                                    bias=b_t[:, m : m + 1],
                                    scale=1.0 / 16.0,
                                )
                        nc.sync.dma_start(
                            out_d.ap()[
                                img,
                                m * 128 : (m + 1) * 128,
                                qb * 32 : (qb + 1) * 32,
                                :,
                            ],
                            out_sb[:],
                        )

            for img, m in units:
                stage_c(stage_ab(img, m), img, m)

    nc.compile()
    return nc


def _prep_weights(w):
    """w (256,256,3,3) -> [c_local, (stat idx, o_local)] fp32."""
    wt = np.empty((128, NSTAT_TOT, 128), dtype=np.float32)
    wt[:, NSTAT, :] = np.eye(128, dtype=np.float32)
    wt[:, NSTAT + 1, :] = 3.0 * np.eye(128, dtype=np.float32)
    for m in range(NM):
        for pi, (_, _, taps) in enumerate(PLANES):
            for ti, (_, _, wi, wj) in enumerate(taps):
                for k in range(NK):
                    si = STAT_IDX[(m, pi, ti, k)]
                    sub = w[m * 128 : (m + 1) * 128, k * 128 : (k + 1) * 128, wi, wj]
                    wt[:, si, :] = sub.T
    return np.ascontiguousarray(wt.reshape(128, NSTAT_TOT * 128))


def kernel(x, w, b):
    global _compiled, LAST_RESULTS
    if _compiled is None:
        _compiled = _build()
    nc = _compiled

    x = np.asarray(x, dtype=np.float32)
    w = np.asarray(w, dtype=np.float32)
    b = np.asarray(b, dtype=np.float32)

    import ml_dtypes

    wt = _prep_weights(w).astype(ml_dtypes.bfloat16)
    b2 = np.ascontiguousarray(b.reshape(NM, 128).T)
    xp = np.zeros((IMGS, C, XR, XC), dtype=np.float32)
    xp[:, :, 1 : H + 1, 1 : W + 1] = x
    xp = np.ascontiguousarray(
        xp.reshape(N_CORES, IMG_PER_CORE, NK, 128, XR * XC)
    ).astype(ml_dtypes.bfloat16)

    in_maps = [
        {"xp": xp[core], "wt": wt, "bias": b2} for core in range(N_CORES)
    ]
    try:
        res = run_bass_kernel_spmd(nc, in_maps, list(range(N_CORES)))
    except ModuleNotFoundError:
        import os

        os.environ["BASS_NEVER_TRACE"] = "1"
        res = run_bass_kernel_spmd(nc, in_maps, list(range(N_CORES)))
    LAST_RESULTS = res
    out = np.concatenate([res.results[i]["out"] for i in range(N_CORES)], axis=0)
    return out



# revision 12
# speedup vs baseline: 1.5252x; 1.0163x over previous
"""StyleGAN2 fused upsample2x + 3x3 conv + FIR(1,3,3,1) + bias — TRN2 Bass kernel v2.

Unlike v1 (which folded the FIR into the conv weights, 4x the matmul work),
this version computes the four parity planes of the stride-2 transposed conv
directly (9 taps total across planes -> 4x fewer MACs on TensorE), then applies
the separable FIR (1,3,3,1)/4 per dimension as fused scalar_tensor_tensor ops:

  y parity planes (PSUM, fp32) --ScalarE copy--> bf16 SBUF (plus shifted-by-1
  copies via GPSIMD so every DVE operand stays 4B-aligned => 2x bf16 mode)
  --DVE col FIR--> h planes --DVE row FIR--> quadrants
  --ScalarE scale(1/16)+bias--> interleaved fp32 out --DMA--> HBM.

Data-parallel over batch: 2 images per core, 8 cores.  Matmuls in float32r.
"""

import sys

sys.path.insert(0, "/opt/trn_rl_repo")

import numpy as np

import concourse.bacc as bacc
import concourse.mybir as mybir
import concourse.tile as tile
from concourse.bass_utils import run_bass_kernel_spmd

N_CORES = 8
IMGS = 16
IMG_PER_CORE = IMGS // N_CORES  # 2
C = 256
O = 256
H = W = 64
NK = C // 128  # contraction splits
NM = O // 128  # output-channel splits
XR, XC = H + 3, W + 4  # padded input rows/cols (67, 68)
PW = W + 2  # stored plane width (66)

# (plane, rows, [(du, dv, wi, wj), ...]) in kernel iteration order.
# E-class planes have H+1 rows, O-class (row-shifted storage) H+2.
PLANES = [
    ("Ee", H + 1, [(0, 0, 0, 0), (0, 1, 0, 2), (1, 0, 2, 0), (1, 1, 2, 2)]),
    ("Eo", H + 1, [(0, 0, 0, 1), (1, 0, 2, 1)]),
    ("Oe", H + 2, [(0, 0, 1, 0), (0, 1, 1, 2)]),
    ("Oo", H + 2, [(0, 0, 1, 1)]),
]


def _chunks(rows):
    """7-row PSUM chunks paired into bands of (up to) 14 rows."""
    starts = list(range(0, rows, 7))
    ch = [(s, min(7, rows - s)) for s in starts]
    bands = [ch[i : i + 2] for i in range(0, len(ch), 2)]
    return bands


def _stat_order():
    """Stationary weight order: (m, plane_idx, tap_idx, k) -> flat index."""
    order = []
    for m in range(NM):
        for pi, (_, _, taps) in enumerate(PLANES):
            for ti in range(len(taps)):
                for k in range(NK):
                    order.append((m, pi, ti, k))
    return {key: i for i, key in enumerate(order)}


STAT_IDX = _stat_order()
NSTAT = len(STAT_IDX)  # 36
NSTAT_TOT = NSTAT + 2  # + identity, 3*identity for FIR combine matmuls

_compiled = None
LAST_RESULTS = None


def _build():
    nc = bacc.Bacc(None, target_bir_lowering=False, debug=False)
    dt = mybir.dt
    f32r, f32, bf16 = dt.float32r, dt.float32, dt.bfloat16
    MULT, ADD = mybir.AluOpType.mult, mybir.AluOpType.add

    xp_d = nc.dram_tensor(
        "xp", (IMG_PER_CORE, NK, 128, XR * XC), bf16, kind="ExternalInput"
    )
    wt_d = nc.dram_tensor("wt", (128, NSTAT_TOT * 128), bf16, kind="ExternalInput")
    b_d = nc.dram_tensor("bias", (128, NM), f32, kind="ExternalInput")
    out_d = nc.dram_tensor(
        "out", (IMG_PER_CORE, O, 2 * H, 2 * W), f32, kind="ExternalOutput"
    )

    with tile.TileContext(nc) as tc:
        with (
            tc.tile_pool(name="xpool", bufs=1) as xpool,
            tc.tile_pool(name="wpool", bufs=1) as wpool,
            tc.tile_pool(name="ybpool", bufs=4) as ybpool,
            tc.tile_pool(name="pqpool", bufs=2) as pqpool,
            tc.tile_pool(name="hpool", bufs=2) as hpool,
            tc.tile_pool(name="cpool", bufs=2) as cpool,
            tc.tile_pool(name="qpool", bufs=4) as qpool,
            tc.tile_pool(name="opool", bufs=2) as opool,
            tc.tile_pool(name="psum", bufs=6, space="PSUM") as psum_pool,
            tc.tile_pool(name="qpsum", bufs=2, space="PSUM") as qpsum,
        ):
            wt_t = wpool.tile([128, NSTAT_TOT * 128], bf16, tag="wt")
            b_t = wpool.tile([128, NM], f32, tag="bias")
            xp_t = {}

            def load_xp(img, k, split=False):
                t = xpool.tile([128, XR, XC], bf16, tag=f"xp{img}{k}")
                src = xp_d.ap()[img, k].rearrange("p (h w) -> p h w", h=XR)
                if split:
                    nc.sync.dma_start(t[:, :20, :], src[:, :20, :])
                    nc.sync.dma_start(t[:, 20:, :], src[:, 20:, :])
                else:
                    nc.sync.dma_start(t[:], src)
                xp_t[img, k] = t

            # Minimal working set first: weights for (m0, plane Ee), first xp
            # rows, then the rest.
            nc.sync.dma_start(wt_t[:, : 8 * 128], wt_d.ap()[:, : 8 * 128])
            load_xp(0, 0, split=True)
            nc.sync.dma_start(b_t[:], b_d.ap()[:])
            load_xp(0, 1, split=True)
            nc.sync.dma_start(wt_t[:, 8 * 128 :], wt_d.ap()[:, 8 * 128 :])
            load_xp(1, 0)
            load_xp(1, 1)

            I1 = wt_t[:, NSTAT * 128 : (NSTAT + 1) * 128]
            I3 = wt_t[:, (NSTAT + 1) * 128 : (NSTAT + 2) * 128]

            # Software-pipelined over (img, m) units: stage C of unit u-1 is
            # emitted after stage A+B of unit u so the PE/Scalar FIFOs never
            # stall on the current unit's col-FIR completing.
            units = [
                (img, m) for img in range(IMG_PER_CORE) for m in range(NM)
            ]
            done_h = []

            def stage_ab(img, m):
                    # ---- stage A+B: matmul parity planes, evac, col FIR ----
                    h_t = {}
                    for name, rows, _ in PLANES:
                        h_t[name] = hpool.tile(
                            [128, rows, W], bf16, tag=f"h{name}", name=f"h{name}"
                        )

                    for cls, rows in (("E", H + 1), ("O", H + 2)):
                        pe_i, po_i = (0, 1) if cls == "E" else (2, 3)
                        _, _, pe_taps = PLANES[pe_i]
                        _, _, po_taps = PLANES[po_i]
                        for band in _chunks(rows):
                            r0 = band[0][0]
                            nr = sum(n for _, n in band)
                            psums = {}
                            for pi, taps in ((pe_i, pe_taps), (po_i, po_taps)):
                                pts = [
                                    psum_pool.tile(
                                        [128, n, PW], f32, tag="ps", name="ps"
                                    )
                                    for _, n in band
                                ]
                                n_ops = len(taps) * NK
                                acc = 0
                                for ti, (du, dv, _, _) in enumerate(taps):
                                    for k in range(NK):
                                        si = STAT_IDX[(m, pi, ti, k)]
                                        lhsT = wt_t[:, si * 128 : (si + 1) * 128]
                                        for ci, (cs, cn) in enumerate(band):
                                            rhs = xp_t[img, k][
                                                :,
                                                cs + du : cs + du + cn,
                                                dv : dv + PW,
                                            ]
                                            nc.tensor.matmul(
                                                pts[ci][:],
                                                lhsT,
                                                rhs,
                                                start=(acc == 0),
                                                stop=(acc == n_ops - 1),
                                            )
                                        acc += 1
                                psums[pi] = pts

                            # evac: n copies only (ScalarE, PSUM->SBUF bf16)
                            yb = {}
                            for pi in (pe_i, po_i):
                                n_t = ybpool.tile([128, nr, PW], bf16, tag="ybn")
                                ro = 0
                                for ci, (cs, cn) in enumerate(band):
                                    nc.scalar.copy(
                                        n_t[:, ro : ro + cn, :], psums[pi][ci][:]
                                    )
                                    ro += cn
                                yb[pi] = n_t

                            ne = yb[pe_i]
                            no = yb[po_i]
                            # col FIR (unnormalized x4), all on DVE:
                            #   he = 3*ne[0] + ne[1] + 3*no[1] + no[0]
                            #      = (3*ne[0]+no[0]) + (3*no[1]+ne[1])
                            #   ho = 3*ne[1] + ne[0] + 3*no[1] + no[2]
                            #      = 3*(ne[1]+no[1]) + (ne[0]+no[2])
                            he = h_t["Ee" if cls == "E" else "Oe"]
                            ho = h_t["Eo" if cls == "E" else "Oo"]
                            def pe_fir(dst, taps4):
                                # FIR combine as identity-matmul accumulation
                                # on TensorE + ScalarE PSUM evac
                                ro = 0
                                for ci, (cs, cn) in enumerate(band):
                                    hp = qpsum.tile(
                                        [128, 8, W], f32, tag="qp", name="qp"
                                    )
                                    for j, (ww, src, dv) in enumerate(taps4):
                                        nc.tensor.matmul(
                                            hp[:, 0:cn, :],
                                            ww,
                                            src[:, ro : ro + cn, dv : dv + W],
                                            start=(j == 0),
                                            stop=(j == 3),
                                        )
                                    nc.scalar.copy(
                                        dst[:, r0 + ro : r0 + ro + cn, :],
                                        hp[:, 0:cn, :],
                                    )
                                    ro += cn

                            pe_he = cls == "E" and r0 == 28
                            pe_ho = pe_he or (cls == "O" and r0 == 28)
                            # STT is 1x-only on DVE; route scales through
                            # tensor_scalar (4x) and keep TTs 2x where the
                            # operand byte-alignment allows (even col offsets).
                            if pe_he:
                                pe_fir(he, ((I3, ne, 0), (I1, ne, 1),
                                            (I3, no, 1), (I1, no, 0)))
                            else:
                                ne3 = pqpool.tile(
                                    [128, nr, PW], bf16, tag="n3"
                                )
                                A_ = pqpool.tile([128, nr, W], bf16, tag="Ac")
                                B_ = pqpool.tile([128, nr, W], bf16, tag="Bc")
                                nc.vector.tensor_scalar_mul(ne3[:], ne[:], 3.0)
                                nc.vector.tensor_tensor(
                                    A_[:], ne3[:, :, 0:W], no[:, :, 0:W], ADD
                                )
                                nc.vector.scalar_tensor_tensor(
                                    B_[:], no[:, :, 1 : W + 1], 3.0,
                                    ne[:, :, 1 : W + 1], MULT, ADD
                                )
                                nc.vector.tensor_tensor(
                                    he[:, r0 : r0 + nr, :], A_[:], B_[:], ADD
                                )
                            if pe_ho:
                                pe_fir(ho, ((I1, ne, 0), (I3, ne, 1),
                                            (I3, no, 1), (I1, no, 2)))
                            else:
                                U_ = pqpool.tile([128, nr, W], bf16, tag="Uc")
                                T_ = pqpool.tile([128, nr, W], bf16, tag="Tc")
                                T3 = pqpool.tile([128, nr, W], bf16, tag="T3")
                                nc.vector.tensor_tensor(
                                    U_[:], ne[:, :, 0:W], no[:, :, 2 : W + 2],
                                    ADD
                                )
                                nc.vector.tensor_tensor(
                                    T_[:], ne[:, :, 1 : W + 1],
                                    no[:, :, 1 : W + 1], ADD
                                )
                                nc.vector.tensor_scalar_mul(T3[:], T_[:], 3.0)
                                nc.vector.tensor_tensor(
                                    ho[:, r0 : r0 + nr, :], T3[:], U_[:], ADD
                                )
                    return h_t

            def stage_c(h_t, img, m):
                    # ---- stage C: row FIR + final ----
                    # even out-rows (alpha=0) on DVE, odd (alpha=1) on TensorE
                    for qb in range(4):
                        u0 = qb * 16
                        out_sb = opool.tile([128, 32, 2 * W], f32)
                        for cj, cp in enumerate(("e", "o")):
                            A = h_t["E" + cp]
                            B = h_t["O" + cp]
                            # oE = (3A[u] + A[u+1]) + (3B[u+1] + B[u])
                            A3 = cpool.tile([128, 16, W], bf16, tag="A3")
                            B3 = cpool.tile([128, 16, W], bf16, tag="B3")
                            P_ = cpool.tile([128, 16, W], bf16, tag="Pr")
                            Q_ = cpool.tile([128, 16, W], bf16, tag="Qr")
                            nc.vector.tensor_scalar_mul(
                                A3[:], A[:, u0 : u0 + 16, :], 3.0
                            )
                            nc.vector.tensor_scalar_mul(
                                B3[:], B[:, u0 + 1 : u0 + 17, :], 3.0
                            )
                            nc.vector.tensor_tensor(
                                P_[:], A3[:], A[:, u0 + 1 : u0 + 17, :], ADD
                            )
                            nc.vector.tensor_tensor(
                                Q_[:], B3[:], B[:, u0 : u0 + 16, :], ADD
                            )
                            oE = qpool.tile([128, 16, W], bf16, tag="oE")
                            nc.vector.tensor_tensor(oE[:], P_[:], Q_[:], ADD)
                            nc.scalar.activation(
                                out_sb[:, 0::2, cj::2],
                                oE[:],
                                mybir.ActivationFunctionType.Identity,
                                bias=b_t[:, m : m + 1],
                                scale=1.0 / 16.0,
                            )
                            # oO = A[u] + 3A[u+1] + 3B[u+1] + B[u+2]  (PE psum)
                            for sub in range(2):
                                u1 = u0 + sub * 8
                                qp = qpsum.tile([128, 8, W], f32, tag="qp",
                                                name="qp")
                                nc.tensor.matmul(
                                    qp[:], I1, A[:, u1 : u1 + 8, :],
                                    start=True, stop=False,
                                )
                                nc.tensor.matmul(
                                    qp[:], I3, A[:, u1 + 1 : u1 + 9, :],
                                    start=False, stop=False,
                                )
                                nc.tensor.matmul(
                                    qp[:], I3, B[:, u1 + 1 : u1 + 9, :],
                                    start=False, stop=False,
                                )
                                nc.tensor.matmul(
                                    qp[:], I1, B[:, u1 + 2 : u1 + 10, :],
                                    start=False, stop=True,
                                )
                                nc.scalar.activation(
                                    out_sb[:, sub * 16 + 1 : sub * 16 + 16 : 2,
                                           cj::2],
                                    qp[:],
                                    mybir.ActivationFunctionType.Identity,
                                    bias=b_t[:, m : m + 1],
                                    scale=1.0 / 16.0,
                                )
                        nc.sync.dma_start(
                            out_d.ap()[
                                img,
                                m * 128 : (m + 1) * 128,
                                qb * 32 : (qb + 1) * 32,
                                :,
                            ],
                            out_sb[:],
                        )

            for img, m in units:
                stage_c(stage_ab(img, m), img, m)

    nc.compile()
    return nc


def _prep_weights(w):
    """w (256,256,3,3) -> [c_local, (stat idx, o_local)] fp32."""
    wt = np.empty((128, NSTAT_TOT, 128), dtype=np.float32)
    wt[:, NSTAT, :] = np.eye(128, dtype=np.float32)
    wt[:, NSTAT + 1, :] = 3.0 * np.eye(128, dtype=np.float32)
    for m in range(NM):
        for pi, (_, _, taps) in enumerate(PLANES):
            for ti, (_, _, wi, wj) in enumerate(taps):
                for k in range(NK):
                    si = STAT_IDX[(m, pi, ti, k)]
                    sub = w[m * 128 : (m + 1) * 128, k * 128 : (k + 1) * 128, wi, wj]
                    wt[:, si, :] = sub.T
    return np.ascontiguousarray(wt.reshape(128, NSTAT_TOT * 128))


def kernel(x, w, b):
    global _compiled, LAST_RESULTS
    if _compiled is None:
        _compiled = _build()
    nc = _compiled

    x = np.asarray(x, dtype=np.float32)
    w = np.asarray(w, dtype=np.float32)
    b = np.asarray(b, dtype=np.float32)

    import ml_dtypes

    wt = _prep_weights(w).astype(ml_dtypes.bfloat16)
    b2 = np.ascontiguousarray(b.reshape(NM, 128).T)
    xp = np.zeros((IMGS, C, XR, XC), dtype=np.float32)
    xp[:, :, 1 : H + 1, 1 : W + 1] = x
    xp = np.ascontiguousarray(
        xp.reshape(N_CORES, IMG_PER_CORE, NK, 128, XR * XC)
    ).astype(ml_dtypes.bfloat16)

    in_maps = [
        {"xp": xp[core], "wt": wt, "bias": b2} for core in range(N_CORES)
    ]
    try:
        res = run_bass_kernel_spmd(nc, in_maps, list(range(N_CORES)))
    except ModuleNotFoundError:
        import os

        os.environ["BASS_NEVER_TRACE"] = "1"
        res = run_bass_kernel_spmd(nc, in_maps, list(range(N_CORES)))
    LAST_RESULTS = res
    out = np.concatenate([res.results[i]["out"] for i in range(N_CORES)], axis=0)
    return out

